# revision 1
# baseline (speedup 1.0000x reference)
"""CGNN layer kernel for Trainium2 (8 NeuronCores, SPMD).

Sharding: core c owns batch b = c//2 and receiver-node half i0 = (c%2)*128.
Each core computes its (128, 128) output shard from full-j message passing.

Math (per core, b fixed):
  z[i,j,:]  = W1a x_i + W1b x_j + W1d a_ij + W1c c + b1        (pre-activation)
  s[i,:]    = sum_j mask_j * silu(z[i,j,:])
  aggr      = W2 s + b2 * (#live j)
  u         = silu(W3 [x, aggr] + b3); out = LN(x + W4 u + b4) * gamma + beta

Device layout: z kept as (h=128 partitions, j=256 free) per receiver i.
  - adj term: PE-transpose 4-receiver stacks of adj (j,r)->(r,j), masked evict,
    then K=32 row-tiled matmuls (tile_position) against replicated W1d^T.
  - x_j term: one K=128 matmul vs pre-masked x^T (same operands every i).
  - bias+silu+sum_j: single ACT op (bias port + accum_out).
  - masked-j bias pollution removed in closed form: s -= nm0 * silu(beta_i).

Scheduling notes: walrus gives compute instructions a budget of ONE semaphore
wait, and only waits arising from real data dependencies update Tile's
per-engine clock. The kernel therefore "absorbs" cross-engine production ticks
with tiny 1x1 matmuls that genuinely read one stale element of the producer
tile (into a dedicated PSUM scratch column), so every real matmul needs at
most its single PSUM-recycle wait. All MLP biases are folded into PSUM via
K=1 rank-1 matmuls of host-provided bias ROWS against a ones row, so no ACT
instruction ever waits on a DMA. All PSUM pools live for the whole program so
banks never alias across phases.
"""

import numpy as np
import ml_dtypes
ml_bf16 = ml_dtypes.bfloat16
from contextlib import ExitStack

import concourse.bass as bass
import concourse.bacc as bacc
import concourse.mybir as mybir
import concourse.tile as tile
from concourse.bass_utils import run_bass_kernel_spmd
from concourse.tile_rust import add_dep_helper

B, N, H, R = 4, 256, 128, 32
NI = 128          # receivers per core
NQ = NI // 4      # receiver quads
FP = mybir.dt.float32
BF = mybir.dt.bfloat16
EPS = 1e-5
ALU = mybir.AluOpType
ACTF = mybir.ActivationFunctionType

_cache = {}


def _order(later, earlier):
    a = later.ins if hasattr(later, "ins") else later
    b = earlier.ins if hasattr(earlier, "ins") else earlier
    add_dep_helper(a, b, sync=False, reason="pe order")


def _build_program():
    nc = bacc.Bacc()

    # ---- per-core DRAM parameters ----
    adj = nc.declare_dram_parameter("adj", [NI, N, R], FP, isOutput=False)
    x_all = nc.declare_dram_parameter("x_all", [N, H], FP, isOutput=False)
    xi = nc.declare_dram_parameter("xi", [NI, H], FP, isOutput=False)
    maskf = nc.declare_dram_parameter("maskf", [N], FP, isOutput=False)
    condrep = nc.declare_dram_parameter("condrep", [2 * H, H], FP, isOutput=False)
    w1aT = nc.declare_dram_parameter("w1aT", [H, H], FP, isOutput=False)
    w1bT = nc.declare_dram_parameter("w1bT", [H, H], BF, isOutput=False)
    w1cT = nc.declare_dram_parameter("w1cT", [2 * H, H], FP, isOutput=False)
    w1dTrep = nc.declare_dram_parameter("w1dTrep", [H, H], BF, isOutput=False)
    w2T = nc.declare_dram_parameter("w2T", [H, H], FP, isOutput=False)
    w3aT = nc.declare_dram_parameter("w3aT", [H, H], FP, isOutput=False)
    w3bT = nc.declare_dram_parameter("w3bT", [H, H], FP, isOutput=False)
    w4T = nc.declare_dram_parameter("w4T", [H, H], FP, isOutput=False)
    b1row = nc.declare_dram_parameter("b1row", [1, H], FP, isOutput=False)
    b2row = nc.declare_dram_parameter("b2row", [1, H], FP, isOutput=False)
    b3row = nc.declare_dram_parameter("b3row", [1, H], FP, isOutput=False)
    b4row = nc.declare_dram_parameter("b4row", [1, H], FP, isOutput=False)
    onesrow = nc.declare_dram_parameter("onesrow", [1, NI], FP, isOutput=False)
    identp = nc.declare_dram_parameter("identp", [H, H], FP, isOutput=False)
    gamma_rep = nc.declare_dram_parameter("gamma_rep", [H, H], FP, isOutput=False)
    beta_rep = nc.declare_dram_parameter("beta_rep", [H, H], FP, isOutput=False)
    out = nc.declare_dram_parameter("out", [NI, H], FP, isOutput=True)

    with ExitStack() as ctx:
        tc = ctx.enter_context(tile.TileContext(nc))
        const = ctx.enter_context(tc.tile_pool(name="const", bufs=1))
        persist = ctx.enter_context(tc.tile_pool(name="persist", bufs=1))
        work = ctx.enter_context(tc.tile_pool(name="work", bufs=2))
        adjbuf = ctx.enter_context(tc.tile_pool(name="adjbuf", bufs=3))
        scr = ctx.enter_context(tc.tile_pool(name="scr", bufs=3))
        # PSUM: 2 (setup/epilogue) + 4 (z) + 2 (adjT)
        pep = ctx.enter_context(tc.tile_pool(name="pep", bufs=2, space="PSUM"))
        pz = ctx.enter_context(tc.tile_pool(name="pz", bufs=4, space="PSUM"))
        pt = ctx.enter_context(tc.tile_pool(name="pt", bufs=2, space="PSUM"))

        cload_tiles = []

        def cload(ap, shape, tag, dt=FP):
            if not isinstance(ap, bass.AP):
                ap = ap[:]
            t = const.tile(shape, dt, tag=tag, name=tag)
            nc.sync.dma_start(out=t, in_=ap)
            cload_tiles.append(t)
            return t

        ident_sb = cload(identp, [H, H], "ident")
        w1aT_sb = cload(w1aT, [H, H], "w1aT")
        w1bT_sb = cload(w1bT, [H, H], "w1bT", dt=BF)
        w1cT_sb0 = cload(w1cT[0:H, :], [H, H], "w1cT0")
        w1cT_sb1 = cload(w1cT[H:2 * H, :], [H, H], "w1cT1")
        w1dTrep_sb = cload(w1dTrep, [H, H], "w1dTrep", dt=BF)
        w2T_sb = cload(w2T, [H, H], "w2T")
        w3aT_sb = cload(w3aT, [H, H], "w3aT")
        w3bT_sb = cload(w3bT, [H, H], "w3bT")
        w4T_sb = cload(w4T, [H, H], "w4T")
        condrep_sb0 = cload(condrep[0:H, :], [H, H], "condrep0")
        condrep_sb1 = cload(condrep[H:2 * H, :], [H, H], "condrep1")
        b1r_sb = cload(b1row, [1, H], "b1r")
        b2r_sb = cload(b2row, [1, H], "b2r")
        b3r_sb = cload(b3row, [1, H], "b3r")
        b4r_sb = cload(b4row, [1, H], "b4r")
        ones_sb = cload(onesrow, [1, NI], "onesr")
        xi_sb = cload(xi, [NI, H], "xi")
        xall_sb0 = cload(x_all[0:H, :], [H, H], "xall0")
        xall_sb1 = cload(x_all[H:N, :], [H, H], "xall1")
        gamma_sb = cload(gamma_rep, [H, H], "gamma_rep")
        beta_sb = cload(beta_rep, [H, H], "beta_rep")

        # mask broadcast to all partitions: (128, 256)
        maskrep = persist.tile([H, N], FP, tag="maskrep", name="maskrep")
        maskf_ap = maskf[:]
        mask_bcast = bass.AP(tensor=maskf_ap.tensor, offset=maskf_ap.offset,
                             ap=[[0, H]] + list(maskf_ap.ap))
        nc.sync.dma_start(out=maskrep, in_=mask_bcast)

        # per-partition live-count and masked-out-count of senders
        msum = persist.tile([H, 1], FP, tag="msum", name="msum")
        mrow_scr = persist.tile([H, N], FP, tag="mrow_scr", name="mrow_scr")
        nc.vector.tensor_scalar(mrow_scr, maskrep, 1.0, None,
                                ALU.mult, ALU.add, accum_out=msum)
        nm0col = persist.tile([H, 1], FP, tag="nm0col", name="nm0col")
        nc.vector.tensor_scalar(nm0col, msum, -1.0, float(N), ALU.mult, ALU.add)
        # msum replicated as a row (all partitions of msum hold the same value)
        msum_row = persist.tile([1, NI], FP, tag="msum_row", name="msum_row")
        nc.vector.tensor_scalar(msum_row, ones_sb, msum[0:1, 0:1], None,
                                ALU.mult)

        xTm = persist.tile([H, N], BF, tag="xTm", name="xTm")
        xTi = persist.tile([H, NI], FP, tag="xTi", name="xTi")
        ACb = persist.tile([H, NI], FP, tag="ACb", name="ACb")
        siluAC = persist.tile([H, NI], FP, tag="siluAC", name="siluAC")
        korr = persist.tile([H, NI], FP, tag="korr", name="korr")
        S_raw = persist.tile([H, NI], FP, tag="S_raw", name="S_raw")

        # ---- setup: x transposes, ACb ----
        for half, xall_h in ((0, xall_sb0), (1, xall_sb1)):
            pxt = pep.tile([H, H], FP, tag="ps", name="pxt")
            nc.tensor.transpose(pxt, xall_h, ident_sb)
            nc.vector.scalar_tensor_tensor(
                out=xTm[:, half * H:(half + 1) * H], in0=pxt, scalar=1.0,
                in1=maskrep[:, half * H:(half + 1) * H],
                op0=ALU.mult, op1=ALU.mult)

        pxi = pep.tile([H, H], FP, tag="ps", name="pxi")
        nc.tensor.transpose(pxi, xi_sb, ident_sb)
        nc.vector.tensor_copy(xTi, pxi)

        # ACb = W1a x_i + W1c c + b1  -> (128 h, 128 i)
        pA = pep.tile([H, NI], FP, tag="ps", name="pA")
        nc.tensor.matmul(pA, lhsT=w1aT_sb, rhs=xTi, start=True, stop=False)
        nc.tensor.matmul(pA, lhsT=w1cT_sb0, rhs=condrep_sb0,
                         start=False, stop=False)
        nc.tensor.matmul(pA, lhsT=w1cT_sb1, rhs=condrep_sb1,
                         start=False, stop=False)
        nc.tensor.matmul(pA, lhsT=b1r_sb, rhs=ones_sb,
                         start=False, stop=True)
        nc.scalar.activation(ACb, pA, ACTF.Copy)

        # korr[h,i] = nm0 * silu(ACb[h,i])
        nc.scalar.activation(siluAC, ACb, ACTF.Silu)
        nc.vector.tensor_scalar(korr, siluAC, nm0col, None, ALU.mult)

        # ---- main loop over receiver quads ----
        stacks = persist.tile([H, NQ, 2, 4, R], FP, tag="stacks",
                              name="stacks")
        for q in range(NQ):
            st0 = stacks[:, q, 0]
            st1 = stacks[:, q, 1]
            for jt, st, eng in ((0, st0, nc.sync), (1, st1, nc.scalar)):
                asrc = adj[4 * q:4 * q + 4, jt * H:(jt + 1) * H, :]
                eng.dma_start(out=st, in_=asrc.rearrange("g j r -> j g r"))

            ptile = pt.tile([H, N], FP, tag="ptile", name="ptile")
            nc.tensor.transpose(
                ptile[:, 0:H], st0.rearrange("j g r -> j (g r)"), ident_sb)
            nc.tensor.transpose(
                ptile[:, H:N], st1.rearrange("j g r -> j (g r)"), ident_sb)

            atile = adjbuf.tile([H, N], BF, tag="atile", name="atile")
            nc.vector.scalar_tensor_tensor(
                out=atile, in0=ptile, scalar=1.0, in1=maskrep,
                op0=ALU.mult, op1=ALU.mult)

            zts = []
            for g in range(4):
                zt = pz.tile([H, N], FP, tag="zt", name="zt")
                nc.tensor.matmul(zt, lhsT=w1bT_sb, rhs=xTm,
                                 start=True, stop=False)
                zts.append(zt)
            for g in range(4):
                nc.tensor.matmul(
                    zts[g], lhsT=w1dTrep_sb[32 * g:32 * g + 32, :],
                    rhs=atile[32 * g:32 * g + 32, :],
                    start=False, stop=True, tile_position=(32 * g, 0))
            for g in range(4):
                li = 4 * q + g
                sct = scr.tile([H, N], BF, tag="sct", name="sct")
                nc.scalar.activation(sct, zts[g], ACTF.Silu,
                                     bias=ACb[:, li:li + 1])
                sink = scr.tile([H, N], BF, tag="sink", name="sink")
                nc.vector.tensor_scalar(sink, sct, 1.0, None, ALU.mult,
                                        ALU.add, accum_out=S_raw[:, li:li + 1])

        # ---- epilogue ----
        S_true = persist.tile([H, NI], FP, tag="S_true", name="S_true")
        nc.vector.scalar_tensor_tensor(out=S_true, in0=S_raw, scalar=0.0,
                                       in1=korr, op0=ALU.add,
                                       op1=ALU.subtract)
        # aggr = W2 s + b2 * live_count
        pa = pep.tile([H, NI], FP, tag="ps", name="pa")
        nc.tensor.matmul(pa, lhsT=w2T_sb, rhs=S_true, start=True, stop=False)
        nc.tensor.matmul(pa, lhsT=b2r_sb, rhs=msum_row, start=False,
                         stop=True)
        aggrT = work.tile([H, NI], FP, tag="aggrT", name="aggrT")
        nc.scalar.activation(aggrT, pa, ACTF.Copy)

        pu = pep.tile([H, NI], FP, tag="ps", name="pu")
        nc.tensor.matmul(pu, lhsT=w3aT_sb, rhs=xTi, start=True, stop=False)
        nc.tensor.matmul(pu, lhsT=w3bT_sb, rhs=aggrT, start=False, stop=False)
        nc.tensor.matmul(pu, lhsT=b3r_sb, rhs=ones_sb, start=False,
                         stop=True)
        u_sb = work.tile([H, NI], FP, tag="u_sb", name="u_sb")
        nc.scalar.activation(u_sb, pu, ACTF.Silu)

        pupd = pep.tile([H, NI], FP, tag="ps", name="pupd")
        nc.tensor.matmul(pupd, lhsT=w4T_sb, rhs=u_sb, start=True, stop=False)
        nc.tensor.matmul(pupd, lhsT=b4r_sb, rhs=ones_sb, start=False,
                         stop=True)
        updT = work.tile([H, NI], FP, tag="updT", name="updT")
        nc.scalar.activation(updT, pupd, ACTF.Copy)

        py = pep.tile([NI, H], FP, tag="ps", name="py")
        nc.tensor.transpose(py, updT, ident_sb)

        y_sb = work.tile([NI, H], FP, tag="y_sb", name="y_sb")
        rowsum = work.tile([NI, 1], FP, tag="rowsum", name="rowsum")
        nc.vector.scalar_tensor_tensor(out=y_sb, in0=py, scalar=0.0,
                                       in1=xi_sb, op0=ALU.add, op1=ALU.add,
                                       accum_out=rowsum)
        negmu = work.tile([NI, 1], FP, tag="negmu", name="negmu")
        nc.vector.tensor_scalar(negmu, rowsum, -1.0 / H, None, ALU.mult)

        ysq = work.tile([NI, H], FP, tag="ysq", name="ysq")
        sumsq = work.tile([NI, 1], FP, tag="sumsq", name="sumsq")
        nc.vector.scalar_tensor_tensor(out=ysq, in0=y_sb, scalar=0.0,
                                       in1=y_sb, op0=ALU.add, op1=ALU.mult,
                                       accum_out=sumsq)
        # var + eps = sumsq/H - mu^2 + eps
        ex2 = work.tile([NI, 1], FP, tag="ex2", name="ex2")
        nc.vector.tensor_scalar(ex2, sumsq, 1.0 / H, float(EPS),
                                ALU.mult, ALU.add)
        musq = work.tile([NI, 1], FP, tag="musq", name="musq")
        nc.vector.scalar_tensor_tensor(out=musq, in0=negmu, scalar=0.0,
                                       in1=negmu, op0=ALU.add, op1=ALU.mult)
        vare = work.tile([NI, 1], FP, tag="vare", name="vare")
        nc.vector.scalar_tensor_tensor(out=vare, in0=ex2, scalar=0.0,
                                       in1=musq, op0=ALU.add,
                                       op1=ALU.subtract)
        sd = work.tile([NI, 1], FP, tag="sd", name="sd")
        nc.scalar.activation(sd, vare, ACTF.Sqrt)
        rstd = work.tile([NI, 1], FP, tag="rstd", name="rstd")
        nc.vector.reciprocal(rstd, sd)

        yn = work.tile([NI, H], FP, tag="yn", name="yn")
        nc.vector.tensor_scalar(yn, y_sb, negmu, rstd, ALU.add, ALU.mult)
        yg = work.tile([NI, H], FP, tag="yg", name="yg")
        nc.vector.scalar_tensor_tensor(out=yg, in0=yn, scalar=0.0,
                                       in1=gamma_sb, op0=ALU.add,
                                       op1=ALU.mult)
        yfin = work.tile([NI, H], FP, tag="yfin", name="yfin")
        nc.vector.scalar_tensor_tensor(out=yfin, in0=yg, scalar=0.0,
                                       in1=beta_sb, op0=ALU.add,
                                       op1=ALU.add)
        nc.sync.dma_start(out=out[:], in_=yfin)

    nc.finalize()
    return nc


def _get_program():
    if "nc" not in _cache:
        _cache["nc"] = _build_program()
    return _cache["nc"]


def kernel(x, adj_dist, mask, cond_vec, W1, b1, W2, b2, W3, b3, W4, b4,
           gamma, beta):
    x = np.asarray(x, dtype=np.float32)
    adj_dist = np.asarray(adj_dist, dtype=np.float32)
    mask_np = np.asarray(mask)
    cond_vec = np.asarray(cond_vec, dtype=np.float32)
    W1 = np.asarray(W1, dtype=np.float32)
    W2 = np.asarray(W2, dtype=np.float32)
    W3 = np.asarray(W3, dtype=np.float32)
    W4 = np.asarray(W4, dtype=np.float32)

    def c(a):
        return np.ascontiguousarray(a, dtype=np.float32)

    shared = dict(
        w1aT=c(W1[:, 0:H].T),
        w1bT=np.ascontiguousarray(W1[:, H:2 * H].T.astype(ml_bf16)),
        w1cT=c(W1[:, 2 * H + R:].T),
        w1dTrep=np.ascontiguousarray(
            np.tile(W1[:, 2 * H:2 * H + R].T, (4, 1)).astype(ml_bf16)),
        w2T=c(W2.T), w3aT=c(W3[:, 0:H].T), w3bT=c(W3[:, H:2 * H].T),
        w4T=c(W4.T),
        b1row=c(np.asarray(b1).reshape(1, H)),
        b2row=c(np.asarray(b2).reshape(1, H)),
        b3row=c(np.asarray(b3).reshape(1, H)),
        b4row=c(np.asarray(b4).reshape(1, H)),
        onesrow=c(np.ones((1, NI))),
        identp=c(np.eye(H)),
        gamma_rep=c(np.tile(np.asarray(gamma)[None, :], (H, 1))),
        beta_rep=c(np.tile(np.asarray(beta)[None, :], (H, 1))),
    )

    in_maps = []
    for core in range(8):
        b, ih = core // 2, core % 2
        i0 = ih * NI
        m = dict(shared)
        m["adj"] = c(adj_dist[b, i0:i0 + NI])
        m["x_all"] = c(x[b])
        m["xi"] = c(x[b, i0:i0 + NI])
        m["maskf"] = c(mask_np[b].astype(np.float32))
        m["condrep"] = c(np.tile(cond_vec[b][:, None], (1, H)))
        in_maps.append(m)

    nc = _get_program()
    _cache["in_maps"] = in_maps
    res = run_bass_kernel_spmd(nc, in_maps, list(range(8)))

    out_full = np.empty((B, N, H), dtype=np.float32)
    for core in range(8):
        b, ih = core // 2, core % 2
        out_full[b, ih * NI:(ih + 1) * NI] = res.results[core]["out"]
    return out_full



# revision 22
# speedup vs baseline: 2.3137x; 2.3137x over previous
"""CGNN layer kernel for Trainium2 (8 NeuronCores, SPMD).

Sharding: core c owns batch b = c//2 and receiver-node half i0 = (c%2)*128.
Each core computes its (128, 128) output shard from full-j message passing.

Key ideas vs naive:
  - j (sender) axis is COMPACTED on the host to live senders only (mask==1),
    padded to Npad (multiple of 8). All per-edge work shrinks ~2x.
  - adj is pre-transposed and bf16-cast on the host into per-chunk matmul
    rhs blocks [32+NR, NR*Npad]: rows 0..31 hold adj^T (r-major), rows
    32..32+NR-1 are one-hot rows that inject the per-receiver bias column
    ACb (= W1a x_i + W1c c + b1) through the same matmul (lhsT rows stitched
    on device from computed ACbT). No PE transposes in the main loop.
  - Main loop processes NR receivers per 512-col PSUM bank chunk:
      PE:   z = [W1d; ACbT_rows]^T @ rhs_chunk   (K=32+NR)
      base (W1b x_j, shared across i): either a second K=128 matmul with a
        stride-0 broadcast rhs (PE-mode) or a DVE in-place PSUM add against
        a broadcast AP of the precomputed base row-block (DVE-mode);
        the PE/DVE split is tunable to balance engine load.
      ACT:  sct = Silu(z) chunked (one instr per 512 cols), bf16
      DVE:  segmented tensor_reduce (axis=X) sums over j per receiver.
  - Padded j columns contribute silu(ACb) each; subtracted in closed form
    via korr = (Npad - live) * silu(ACb).
  - Consts ride in 4 packed DMA blobs; adj rhs in 8 parallel slab DMAs.
"""

import math
import numpy as np
import ml_dtypes
ml_bf16 = ml_dtypes.bfloat16
from contextlib import ExitStack

import concourse.bass as bass
import concourse.bacc as bacc
import concourse.mybir as mybir
import concourse.tile as tile
from concourse.bass_utils import run_bass_kernel_spmd
from concourse.tile_rust import add_dep_helper


def _dep(later, earlier, sync=True):
    a = later.ins if hasattr(later, "ins") else later
    b = earlier.ins if hasattr(earlier, "ins") else earlier
    add_dep_helper(a, b, sync=sync, reason="manual dep")

B, N, H, R = 4, 256, 128, 32
NI = 128          # receivers per core
FP = mybir.dt.float32
BF = mybir.dt.bfloat16
EPS = 1e-5
ALU = mybir.AluOpType
ACTF = mybir.ActivationFunctionType
AXL = mybir.AxisListType

_cache = {}

# fraction of chunks whose base-add runs on PE (rest on DVE); tuned on trace
PE_BASE_MOD = 3
PE_BASE_CNT = 1   # chunks with (c % PE_BASE_MOD) < PE_BASE_CNT use PE-mode
DBG_SKIP = set()  # debug: {'stitch','loop','pemode','dveadd','reduce','epi'}


def _bcast_ap(t_ap, nrep):
    """Insert a stride-0 dim after the partition dim: [P, W] -> [P, nrep, W]."""
    ap = list(t_ap.ap)
    new = [list(ap[0]), [0, nrep]] + [list(d) for d in ap[1:]]
    return bass.AP(tensor=t_ap.tensor, offset=t_ap.offset, ap=new)


def _build_program(npad, nr, nc_chunks, nslab, chunks_per_slab):
    KB = 32 + nr
    W = nr * npad                       # cols per chunk (<= 512)
    nfull = NI // nr                    # chunks fully covered by real receivers
    rem = NI - nfull * nr               # receivers in ragged tail chunk
    swidth = nc_chunks * nr             # S columns incl phantom slots

    nc = bacc.Bacc()

    # ---- DRAM parameters ----
    # fp32 blobs
    CCOLS = 5 * H + 3                   # w1aT w1cT0 w1cT1 ident xiT cond0 cond1 nm0
    ECOLS = 7 * H                       # w2T w3aT w3bT w4T xi gamma beta
    blobc = nc.declare_dram_parameter("blobc", [H, CCOLS], FP, isOutput=False)
    blobe = nc.declare_dram_parameter("blobe", [H, ECOLS], FP, isOutput=False)
    blobr = nc.declare_dram_parameter("blobr", [1, 6 * H + 1], FP, isOutput=False)
    blobbf = nc.declare_dram_parameter("blobbf", [H, H + npad], BF, isOutput=False)
    lhs_top = nc.declare_dram_parameter("lhs_top", [32, nc_chunks * H], BF,
                                        isOutput=False)
    rhs = nc.declare_dram_parameter("rhs", [nc_chunks, KB, W], BF,
                                    isOutput=False)
    out = nc.declare_dram_parameter("out", [NI, H], FP, isOutput=True)

    with ExitStack() as ctx:
        tc = ctx.enter_context(tile.TileContext(nc))
        const = ctx.enter_context(tc.tile_pool(name="const", bufs=1))
        persist = ctx.enter_context(tc.tile_pool(name="persist", bufs=1))
        work = ctx.enter_context(tc.tile_pool(name="work", bufs=2))
        sctp = ctx.enter_context(tc.tile_pool(name="sctp", bufs=3))
        pep = ctx.enter_context(tc.tile_pool(name="pep", bufs=2, space="PSUM"))
        pz = ctx.enter_context(tc.tile_pool(name="pz", bufs=4, space="PSUM"))

        # ---- const tiles (sliced views of packed blobs) ----
        cblob = const.tile([H, CCOLS], FP, tag="cblob", name="cblob")
        nc.sync.dma_start(out=cblob, in_=blobc[:])
        w1aT_sb = cblob[:, 0:H]
        w1cT0_sb = cblob[:, H:2 * H]
        w1cT1_sb = cblob[:, 2 * H:3 * H]
        ident_sb = cblob[:, 3 * H:4 * H]
        xiT_sb = cblob[:, 4 * H:5 * H]
        cond0_sb = cblob[:, 5 * H:5 * H + 1]
        cond1_sb = cblob[:, 5 * H + 1:5 * H + 2]
        nm0_sb = cblob[:, 5 * H + 2:5 * H + 3]

        eblob = const.tile([H, ECOLS], FP, tag="eblob", name="eblob")
        nc.scalar.dma_start(out=eblob, in_=blobe[:])
        w2T_sb = eblob[:, 0:H]
        w3aT_sb = eblob[:, H:2 * H]
        w3bT_sb = eblob[:, 2 * H:3 * H]
        w4T_sb = eblob[:, 3 * H:4 * H]
        xi_sb = eblob[:, 4 * H:5 * H]
        gamma_sb = eblob[:, 5 * H:6 * H]
        beta_sb = eblob[:, 6 * H:7 * H]

        rblob = const.tile([1, 6 * H + 1], FP, tag="rblob", name="rblob")
        nc.sync.dma_start(out=rblob, in_=blobr[:])
        b1r_sb = rblob[:, 0:H]
        b2r_sb = rblob[:, H:2 * H]
        b3r_sb = rblob[:, 2 * H:3 * H]
        b4r_sb = rblob[:, 3 * H:4 * H]
        ones_sb = rblob[:, 4 * H:5 * H]
        liver_sb = rblob[:, 5 * H:6 * H]
        ones11_sb = rblob[:, 6 * H:6 * H + 1]

        bfblob = const.tile([H, H + npad], BF, tag="bfblob", name="bfblob")
        nc.gpsimd.dma_start(out=bfblob, in_=blobbf[:])
        w1bT_bf = bfblob[:, 0:H]
        xTm_bf = bfblob[:, H:H + npad]

        # LHS stitched tile: rows 0..31 w1dT (DMA), rows 32.. ACbT (device)
        LHS = const.tile([KB, nc_chunks * H], BF, tag="LHS", name="LHS")
        nc.gpsimd.dma_start(out=LHS[0:32, :], in_=lhs_top[:])

        # RHS slabs
        slabs = []
        slab_engs = [nc.sync, nc.scalar, nc.gpsimd]
        for s in range(nslab):
            c0 = s * chunks_per_slab
            c1 = min(nc_chunks, c0 + chunks_per_slab)
            ns = c1 - c0
            st = const.tile([KB, ns, W], BF, tag=f"slab{s}", name=f"slab{s}")
            src = rhs[c0:c1].rearrange("c k w -> k c w")
            slab_engs[s % len(slab_engs)].dma_start(out=st, in_=src)
            slabs.append((c0, st))

        # ---- setup chain ----
        # trow = (W1c c + b1) as [1, H]
        ptrow = pep.tile([1, H], FP, tag="ps", name="ptrow")
        nc.tensor.matmul(ptrow, lhsT=cond0_sb, rhs=w1cT0_sb,
                         start=True, stop=False)
        nc.tensor.matmul(ptrow, lhsT=cond1_sb, rhs=w1cT1_sb,
                         start=False, stop=False)
        nc.tensor.matmul(ptrow, lhsT=ones11_sb, rhs=b1r_sb,
                         start=False, stop=True)
        trow_sb = persist.tile([1, H], FP, tag="trow", name="trow")
        nc.scalar.activation(trow_sb, ptrow, ACTF.Copy)

        # ACb[h, i] = W1a x_i + trow
        pACb = pep.tile([H, NI], FP, tag="ps", name="pACb")
        nc.tensor.matmul(pACb, lhsT=w1aT_sb, rhs=xiT_sb,
                         start=True, stop=False)
        nc.tensor.matmul(pACb, lhsT=trow_sb, rhs=ones_sb,
                         start=False, stop=True)
        siluACb = persist.tile([H, NI], FP, tag="siluACb", name="siluACb")
        nc.scalar.activation(siluACb, pACb, ACTF.Silu)
        ACb_f32 = persist.tile([H, NI], FP, tag="ACb_f32", name="ACb_f32")
        nc.scalar.activation(ACb_f32, pACb, ACTF.Copy)

        # korr[h, i] = (Npad - live) * silu(ACb)
        korr = persist.tile([H, NI], FP, tag="korr", name="korr")
        nc.vector.tensor_scalar(korr, siluACb, nm0_sb, None, ALU.mult)

        # ACbT via PE transpose -> bf16
        pT = pep.tile([NI, H], FP, tag="ps", name="pT")
        nc.tensor.transpose(pT, ACb_f32, ident_sb)
        ACbT_bf = persist.tile([NI, H], BF, tag="ACbT_bf", name="ACbT_bf")
        acbt_wr = nc.scalar.activation(ACbT_bf, pT, ACTF.Copy)

        # base[h, j] = W1b x_j  (masked-compacted), bf16
        pbase = pep.tile([H, npad], FP, tag="ps", name="pbase")
        nc.tensor.matmul(pbase, lhsT=w1bT_bf, rhs=xTm_bf,
                         start=True, stop=True)
        base_bf = persist.tile([H, npad], BF, tag="base_bf", name="base_bf")
        base_wr = nc.scalar.activation(base_bf, pbase, ACTF.Copy)

        # stitch ACbT rows into LHS rows 32..32+nr-1 (sbuf->sbuf DMA)
        ms_inst = None
        if rem:
            # zero tail-chunk bias rows first (memset must start at part 32)
            ms_inst = nc.vector.memset(LHS[32:32 + nr,
                                           nfull * H:(nfull + 1) * H], 0.0)
        stitch_dmas = []
        if 'stitch' not in DBG_SKIP:
            # bounce via DRAM: partition->free moves need a flat address space
            scratch = nc.dram_tensor("acbt_scratch", [NI, H], BF)
            d0 = nc.sync.dma_start(out=scratch[:], in_=ACbT_bf[0:NI, :])
            _dep(d0, acbt_wr)
            src = scratch[0:nfull * nr, :].rearrange("(c e) h -> e c h", e=nr)
            dst = LHS[32:32 + nr, 0:nfull * H].rearrange("e (c h) -> e c h",
                                                         h=H)
            d1 = nc.sync.dma_start(out=dst, in_=src)
            _dep(d1, d0)
            if ms_inst is not None:
                _dep(d1, ms_inst)
            stitch_dmas.append(d1)
            if rem:
                d2 = nc.sync.dma_start(
                    out=LHS[32:32 + rem, nfull * H:(nfull + 1) * H],
                    in_=scratch[nfull * nr:NI, :])
                _dep(d2, d0)
                if ms_inst is not None:
                    _dep(d2, ms_inst)
                stitch_dmas.append(d2)

        # ---- main loop ----
        S = persist.tile([H, swidth], FP, tag="S", name="S")
        base_bc = _bcast_ap(base_bf[:, :], nr)
        xTm_bc = _bcast_ap(xTm_bf[:, :], nr)

        for c in range(nc_chunks):
            s_idx = c // chunks_per_slab
            c0, st = slabs[s_idx]
            rhs_c = st[:, c - c0, :]
            lhsT_c = LHS[:, c * H:(c + 1) * H]
            pe_mode = (c % PE_BASE_MOD) < PE_BASE_CNT

            if 'loop' in DBG_SKIP:
                continue
            pzc = pz.tile([H, W], FP, tag="pzc", name="pzc")
            if pe_mode and 'pemode' not in DBG_SKIP:
                nc.tensor.matmul(pzc, lhsT=w1bT_bf, rhs=xTm_bc,
                                 start=True, stop=False)
                mmab = nc.tensor.matmul(pzc, lhsT=lhsT_c, rhs=rhs_c,
                                        start=False, stop=True)
            else:
                mmab = nc.tensor.matmul(pzc, lhsT=lhsT_c, rhs=rhs_c,
                                        start=True, stop=True)
            if c == 0:
                for d in stitch_dmas:
                    _dep(mmab, d)
                if 'dveadd' not in DBG_SKIP:
                    nc.vector.tensor_tensor(out=pzc, in0=pzc, in1=base_bc,
                                            op=ALU.add)

            sct = sctp.tile([H, nr, npad], BF, tag="sct", name="sct")
            nc.scalar.activation(sct[:, :, :].rearrange("p a b -> p (a b)"),
                                 pzc, ACTF.Silu)
            if 'reduce' in DBG_SKIP:
                continue
            nc.vector.tensor_reduce(out=S[:, c * nr:(c + 1) * nr],
                                    in_=sct[:, :, :], axis=AXL.X, op=ALU.add)

        # ---- epilogue ----
        S_true = work.tile([H, NI], FP, tag="S_true", name="S_true")
        nc.vector.scalar_tensor_tensor(out=S_true, in0=S[:, 0:NI],
                                       scalar=0.0, in1=korr, op0=ALU.add,
                                       op1=ALU.subtract)
        pa = pep.tile([H, NI], FP, tag="ps", name="pa")
        nc.tensor.matmul(pa, lhsT=w2T_sb, rhs=S_true, start=True, stop=False)
        nc.tensor.matmul(pa, lhsT=b2r_sb, rhs=liver_sb, start=False,
                         stop=True)
        aggrT = work.tile([H, NI], FP, tag="aggrT", name="aggrT")
        nc.scalar.activation(aggrT, pa, ACTF.Copy)

        pu = pep.tile([H, NI], FP, tag="ps", name="pu")
        nc.tensor.matmul(pu, lhsT=w3aT_sb, rhs=xiT_sb, start=True, stop=False)
        nc.tensor.matmul(pu, lhsT=w3bT_sb, rhs=aggrT, start=False, stop=False)
        nc.tensor.matmul(pu, lhsT=b3r_sb, rhs=ones_sb, start=False, stop=True)
        u_sb = work.tile([H, NI], FP, tag="u_sb", name="u_sb")
        nc.scalar.activation(u_sb, pu, ACTF.Silu)

        pupd = pep.tile([H, NI], FP, tag="ps", name="pupd")
        nc.tensor.matmul(pupd, lhsT=w4T_sb, rhs=u_sb, start=True, stop=False)
        nc.tensor.matmul(pupd, lhsT=b4r_sb, rhs=ones_sb, start=False,
                         stop=True)
        updT = work.tile([H, NI], FP, tag="updT", name="updT")
        nc.scalar.activation(updT, pupd, ACTF.Copy)

        py = pep.tile([NI, H], FP, tag="ps", name="py")
        nc.tensor.transpose(py, updT, ident_sb)

        y_sb = work.tile([NI, H], FP, tag="y_sb", name="y_sb")
        rowsum = work.tile([NI, 1], FP, tag="rowsum", name="rowsum")
        nc.vector.scalar_tensor_tensor(out=y_sb, in0=py, scalar=0.0,
                                       in1=xi_sb, op0=ALU.add, op1=ALU.add,
                                       accum_out=rowsum)
        negmu = work.tile([NI, 1], FP, tag="negmu", name="negmu")
        nc.vector.tensor_scalar(negmu, rowsum, -1.0 / H, None, ALU.mult)

        ysq = work.tile([NI, H], FP, tag="ysq", name="ysq")
        sumsq = work.tile([NI, 1], FP, tag="sumsq", name="sumsq")
        nc.vector.scalar_tensor_tensor(out=ysq, in0=y_sb, scalar=0.0,
                                       in1=y_sb, op0=ALU.add, op1=ALU.mult,
                                       accum_out=sumsq)
        ex2 = work.tile([NI, 1], FP, tag="ex2", name="ex2")
        nc.vector.tensor_scalar(ex2, sumsq, 1.0 / H, float(EPS),
                                ALU.mult, ALU.add)
        musq = work.tile([NI, 1], FP, tag="musq", name="musq")
        nc.vector.scalar_tensor_tensor(out=musq, in0=negmu, scalar=0.0,
                                       in1=negmu, op0=ALU.add, op1=ALU.mult)
        vare = work.tile([NI, 1], FP, tag="vare", name="vare")
        nc.vector.scalar_tensor_tensor(out=vare, in0=ex2, scalar=0.0,
                                       in1=musq, op0=ALU.add,
                                       op1=ALU.subtract)
        sd = work.tile([NI, 1], FP, tag="sd", name="sd")
        nc.scalar.activation(sd, vare, ACTF.Sqrt)
        rstd = work.tile([NI, 1], FP, tag="rstd", name="rstd")
        nc.vector.reciprocal(rstd, sd)

        yn = work.tile([NI, H], FP, tag="yn", name="yn")
        nc.vector.tensor_scalar(yn, y_sb, negmu, rstd, ALU.add, ALU.mult)
        yg = work.tile([NI, H], FP, tag="yg", name="yg")
        nc.vector.scalar_tensor_tensor(out=yg, in0=yn, scalar=0.0,
                                       in1=gamma_sb, op0=ALU.add,
                                       op1=ALU.mult)
        yfin = work.tile([NI, H], FP, tag="yfin", name="yfin")
        nc.vector.scalar_tensor_tensor(out=yfin, in0=yg, scalar=0.0,
                                       in1=beta_sb, op0=ALU.add,
                                       op1=ALU.add)
        nc.sync.dma_start(out=out[:], in_=yfin)

    nc.finalize()
    return nc


def _get_program(npad, nr, nc_chunks, nslab, cps):
    key = (npad, nr, nc_chunks, nslab, cps)
    if _cache.get("key") != key:
        _cache["nc"] = _build_program(npad, nr, nc_chunks, nslab, cps)
        _cache["key"] = key
    return _cache["nc"]


def kernel(x, adj_dist, mask, cond_vec, W1, b1, W2, b2, W3, b3, W4, b4,
           gamma, beta):
    x = np.asarray(x, dtype=np.float32)
    adj_dist = np.asarray(adj_dist, dtype=np.float32)
    mask_np = np.asarray(mask)
    cond_vec = np.asarray(cond_vec, dtype=np.float32)
    W1 = np.asarray(W1, dtype=np.float32)
    W2 = np.asarray(W2, dtype=np.float32)
    W3 = np.asarray(W3, dtype=np.float32)
    W4 = np.asarray(W4, dtype=np.float32)

    def c(a):
        return np.ascontiguousarray(a, dtype=np.float32)

    def cb(a):
        return np.ascontiguousarray(np.asarray(a).astype(ml_bf16))

    jidx = [np.nonzero(mask_np[b])[0] for b in range(B)]
    lmax = max(1, max(len(j) for j in jidx))
    npad = ((lmax + 7) // 8) * 8
    nr = max(1, 512 // npad)
    nc_chunks = (NI + nr - 1) // nr
    nslab = min(8, nc_chunks)
    cps = (nc_chunks + nslab - 1) // nslab
    nslab = (nc_chunks + cps - 1) // cps
    KB = 32 + nr
    Wc = nr * npad
    nfull = NI // nr
    rem = NI - nfull * nr

    # shared (per-core-independent) host prep
    w1aT = c(W1[:, 0:H].T)
    w1cT = c(W1[:, 2 * H + R:].T)          # [2H, H]
    w1dT = W1[:, 2 * H:2 * H + R].T        # [32, H]
    blobe_shared = np.concatenate(
        [c(W2.T), c(W3[:, 0:H].T), c(W3[:, H:2 * H].T), c(W4.T)], axis=1)
    gamma_rep = c(np.tile(np.asarray(gamma)[None, :], (H, 1)))
    beta_rep = c(np.tile(np.asarray(beta)[None, :], (H, 1)))
    ident = c(np.eye(H))
    lhs_top = cb(np.tile(w1dT, (1, nc_chunks)))

    onehot = np.zeros((nr, Wc), dtype=np.float32)
    for e in range(nr):
        onehot[e, e * npad:(e + 1) * npad] = 1.0

    in_maps = []
    for core in range(8):
        b, ih = core // 2, core % 2
        i0 = ih * NI
        ji = jidx[b]
        L = len(ji)

        xi = c(x[b, i0:i0 + NI])                        # [NI, H]
        xiT = c(xi.T)                                   # [H, NI]
        xTm = np.zeros((H, npad), dtype=np.float32)
        xTm[:, 0:L] = x[b, ji].T

        cond = cond_vec[b]
        blobc_ = np.concatenate(
            [w1aT, w1cT[0:H], w1cT[H:2 * H], ident, xiT,
             c(cond[0:H][:, None]), c(cond[H:2 * H][:, None]),
             np.full((H, 1), float(npad - L), dtype=np.float32)], axis=1)
        blobe_ = np.concatenate([blobe_shared, xi, gamma_rep, beta_rep],
                                axis=1)
        blobr_ = np.concatenate(
            [c(np.asarray(b1).reshape(1, H)), c(np.asarray(b2).reshape(1, H)),
             c(np.asarray(b3).reshape(1, H)), c(np.asarray(b4).reshape(1, H)),
             np.ones((1, H), dtype=np.float32),
             np.full((1, H), float(L), dtype=np.float32),
             np.ones((1, 1), dtype=np.float32)], axis=1)
        blobbf_ = np.concatenate(
            [W1[:, H:2 * H].T, xTm], axis=1)

        # rhs chunks: [nc_chunks, KB, Wc]
        adjc = np.zeros((NI, npad, R), dtype=np.float32)
        adjc[:, 0:L, :] = adj_dist[b, i0:i0 + NI][:, ji, :]
        rhs_ = np.zeros((nc_chunks, KB, Wc), dtype=np.float32)
        for cc in range(nc_chunks):
            g0 = cc * nr
            ng = min(nr, NI - g0)
            blk = adjc[g0:g0 + ng]                      # [ng, npad, R]
            rhs_[cc, 0:32, 0:ng * npad] = (
                blk.transpose(2, 0, 1).reshape(R, ng * npad))
            rhs_[cc, 32:32 + ng, :] = onehot[0:ng]

        m = dict(
            blobc=c(blobc_), blobe=c(blobe_), blobr=c(blobr_),
            blobbf=cb(blobbf_), lhs_top=lhs_top, rhs=cb(rhs_),
        )
        in_maps.append(m)

    nc = _get_program(npad, nr, nc_chunks, nslab, cps)
    _cache["in_maps"] = in_maps
    res = run_bass_kernel_spmd(nc, in_maps, list(range(8)))

    out_full = np.empty((B, N, H), dtype=np.float32)
    for core in range(8):
        b, ih = core // 2, core % 2
        out_full[b, ih * NI:(ih + 1) * NI] = res.results[core]["out"]
    return out_full
